# revision 5
# baseline (speedup 1.0000x reference)
"""AttnBlock (GroupNorm + single-head self-attention + residual) for TRN2.

8 cores = 2 batches x 4 query-chunks of 1024 tokens.

v3 math restructure ("two-matrix form"): softmax is invariant to per-query
additive constants, and the per-token 1/l commutes with the output
projection.  Folding those out, the whole block needs only two host-
precomputed CxC matrices applied to RAW x:

  scores_ij ~ u_i . x_j   (mod per-i consts), u = a*(Mqk @ h_q + cq),
      Mqk = scale * wk^T wq,  h_q = a*x_q + b (GroupNorm affine)
  A_i = sum_j exp(s_ij) x_j ,  l_i = sum_j exp(s_ij)
  out = x + (WpWv*diag(a)) @ (A/l) + [WpWv b + wp bv + bp]

So K/V/Q/P projections of the token stream disappear (6.98 -> 4.83 GMAC
per core) and the attention matmuls consume x directly in bf16 (half the
DMA, FWL weight loads).  PSUM: A 4 banks (chain over all 32 j-blocks),
scores 2, l 1, epilogue 1 = 8.
"""

import numpy as np
import ml_dtypes
from contextlib import ExitStack

import concourse.bass as bass
import concourse.bacc as bacc
import concourse.tile as tile
from concourse import mybir
from concourse.bass_utils import run_bass_kernel_spmd

F32 = mybir.dt.float32
BF16 = mybir.dt.bfloat16
AL = mybir.AluOpType
AF = mybir.ActivationFunctionType

B = 2
C = 512
N = 4096
NQ = 1024
P = 128
NCC = C // P      # 4
G = 32
EPS = 1e-6
NJB = N // P      # 32 j-blocks of 128 tokens
NIH = NQ // 512   # 2 query halves of 512
SCALE = float(C) ** -0.5
BF = ml_dtypes.bfloat16


def build_nc():
    nc = bacc.Bacc(None, target_bir_lowering=False)

    xh = nc.dram_tensor("xh", [C, N], BF16, kind="ExternalInput")
    xt = nc.dram_tensor("xt", [N, C], BF16, kind="ExternalInput")
    xq = nc.dram_tensor("xq", [C, NQ], F32, kind="ExternalInput")
    mt = nc.dram_tensor("mt", [C, C], BF16, kind="ExternalInput")    # (scale*wk^T wq)^T
    w2t = nc.dram_tensor("w2t", [C, C], BF16, kind="ExternalInput")  # (wp wv)^T
    cvec = nc.dram_tensor("cvec", [C, 2], F32, kind="ExternalInput")  # [cq, wp@bv+bp]
    gaff = nc.dram_tensor("gaff", [C, 2], F32, kind="ExternalInput")
    gm = nc.dram_tensor("gm", [C, G], F32, kind="ExternalInput")     # indicator/16
    gmt = nc.dram_tensor("gmt", [G, C], F32, kind="ExternalInput")   # indicator
    out = nc.dram_tensor("out", [C, NQ], F32, kind="ExternalOutput")

    with tile.TileContext(nc) as tc, ExitStack() as ctx:
        const = ctx.enter_context(tc.tile_pool(name="const", bufs=1))
        xhp = ctx.enter_context(tc.tile_pool(name="xhp", bufs=1))
        xtp = ctx.enter_context(tc.tile_pool(name="xtp", bufs=1))
        xqp = ctx.enter_context(tc.tile_pool(name="xqp", bufs=1))
        wp_ = ctx.enter_context(tc.tile_pool(name="wp", bufs=1))
        utp = ctx.enter_context(tc.tile_pool(name="utp", bufs=1))
        ptp = ctx.enter_context(tc.tile_pool(name="ptp", bufs=3))
        alp = ctx.enter_context(tc.tile_pool(name="alp", bufs=1))
        tmp = ctx.enter_context(tc.tile_pool(name="tmp", bufs=2))
        psA = ctx.enter_context(tc.tile_pool(name="psA", bufs=1, space="PSUM"))
        psS = ctx.enter_context(tc.tile_pool(name="psS", bufs=2, space="PSUM"))
        psL = ctx.enter_context(tc.tile_pool(name="psL", bufs=1, space="PSUM"))
        psE = ctx.enter_context(tc.tile_pool(name="psE", bufs=1, space="PSUM"))

        # ---- tiny constant tables ----
        cvec_sb = []
        gaff_sb = []
        gm_sb = []
        for cc in range(NCC):
            t = const.tile([P, 2], F32, tag=f"cv{cc}", name=f"cv{cc}")
            nc.sync.dma_start(out=t[:], in_=cvec[cc * P:(cc + 1) * P, :])
            cvec_sb.append(t)
            t = const.tile([P, 2], F32, tag=f"ga{cc}", name=f"ga{cc}")
            nc.sync.dma_start(out=t[:], in_=gaff[cc * P:(cc + 1) * P, :])
            gaff_sb.append(t)
            t = const.tile([P, G], F32, tag=f"gm{cc}", name=f"gm{cc}")
            nc.sync.dma_start(out=t[:], in_=gm[cc * P:(cc + 1) * P, :])
            gm_sb.append(t)
        gmt_sb = const.tile([G, C], F32, tag="gmt")
        nc.sync.dma_start(out=gmt_sb[:], in_=gmt[:, :])
        eps_sb = const.tile([G, 1], F32, tag="eps")
        nc.vector.memset(eps_sb[:], EPS)
        ones_row = const.tile([1, P], F32, tag="onesr")
        nc.vector.memset(ones_row[:], 1.0)
        ones_col = const.tile([P, 1], BF16, tag="onesc")
        nc.vector.memset(ones_col[:], 1.0)
        warm_sb = const.tile([P, 512], BF16, tag="warm")
        nc.vector.memset(warm_sb[:], 1.0)

        # ---- u-matrix first on the SP queue (first real matmuls need it),
        # then x (bf16); xq/w2t/xt go on the Activation HWDGE queue ----
        mt_sb = []
        for cc in range(NCC):
            t = wp_.tile([P, C], BF16, tag=f"mt{cc}", name=f"mt{cc}")
            nc.sync.dma_start(out=t[:], in_=mt[cc * P:(cc + 1) * P, :])
            mt_sb.append(t)
        xq_sb = []
        for cc in range(NCC):
            t = xqp.tile([P, NQ], F32, tag=f"xq{cc}", name=f"xq{cc}")
            nc.scalar.dma_start(out=t[:], in_=xq[cc * P:(cc + 1) * P, :])
            xq_sb.append(t)
        w2t_sb = []
        for cc in range(NCC):
            t = wp_.tile([P, C], BF16, tag=f"w2t{cc}", name=f"w2t{cc}")
            nc.scalar.dma_start(out=t[:], in_=w2t[cc * P:(cc + 1) * P, :])
            w2t_sb.append(t)

        # PE warm-up: HAM un-throttles after ~3.4us of sustained activity.
        # 10 free-running matmuls warm the clock, then one matmul paced
        # behind each x-quarter DMA keeps it warm until real work arrives.
        def emit_warm(n, rhs):
            for _ in range(n):
                wps = psS.tile([1, 512], F32, tag="s")
                nc.tensor.matmul(out=wps[:], lhsT=ones_col[:], rhs=rhs,
                                 start=True, stop=True)

        emit_warm(10, warm_sb[:])

        # ---- x (bf16) quarter tiles: GroupNorm stats stream behind DMA ----
        NQT = N // 4
        xh_sb = {}
        for cc in range(NCC):
            for qq in range(4):
                t = xhp.tile([P, NQT], BF16, tag=f"xh{cc}{qq}", name=f"xh{cc}{qq}")
                nc.sync.dma_start(
                    out=t[:],
                    in_=xh[cc * P:(cc + 1) * P, qq * NQT:(qq + 1) * NQT])
                xh_sb[cc, qq] = t
                emit_warm(2, t[:, 0:512])

        xt_sb = []
        for jb in range(NJB):
            t = xtp.tile([P, C], BF16, tag=f"xt{jb}", name=f"xt{jb}")
            nc.scalar.dma_start(out=t[:], in_=xt[jb * P:(jb + 1) * P, :])
            xt_sb.append(t)

        # ---- GroupNorm stats -> per-channel a, b ----
        mus = []
        for cc in range(NCC):
            stats = tmp.tile([P, 8, 6], F32, tag="bst")
            for qq in range(4):
                xv = xh_sb[cc, qq].rearrange("p (s f) -> p s f", f=512)
                for s in range(2):
                    nc.vector.bn_stats(out=stats[:, qq * 2 + s, :], in_=xv[:, s, :])
            mv = tmp.tile([P, 2], F32, tag="mv")
            nc.vector.bn_aggr(out=mv[:], in_=stats[:])
            mu = tmp.tile([P, 2], F32, tag=f"mu{cc}")
            nc.vector.tensor_copy(mu[:, 0:1], mv[:, 0:1])
            nc.vector.scalar_tensor_tensor(
                out=mu[:, 1:2], in0=mv[:, 0:1], scalar=mv[:, 0:1],
                in1=mv[:, 1:2], op0=AL.mult, op1=AL.add)
            mus.append(mu)
        agg_ps = psE.tile([G, 2], F32, tag="e")
        for cc in range(NCC):
            nc.tensor.matmul(out=agg_ps[:], lhsT=gm_sb[cc][:], rhs=mus[cc][:],
                             start=(cc == 0), stop=(cc == NCC - 1))
        eg = tmp.tile([G, 2], F32, tag="eg")
        nc.vector.tensor_copy(eg[:], agg_ps[:])
        msq = tmp.tile([G, 1], F32, tag="msq")
        nc.vector.tensor_mul(msq[:], eg[:, 0:1], eg[:, 0:1])
        grs = tmp.tile([G, 2], F32, tag="grs")
        nc.vector.tensor_copy(grs[:, 0:1], eg[:, 0:1])
        var = tmp.tile([G, 1], F32, tag="var")
        nc.vector.tensor_sub(var[:], eg[:, 1:2], msq[:])
        std = tmp.tile([G, 1], F32, tag="std")
        nc.scalar.activation(out=std[:], in_=var[:], func=AF.Sqrt, bias=eps_sb[:])
        nc.vector.reciprocal(grs[:, 1:2], std[:])

        ab_sb = []
        for cc in range(NCC):
            bc_ps = psE.tile([P, 2], F32, tag="e")
            nc.tensor.matmul(out=bc_ps[:],
                             lhsT=gmt_sb[:, cc * P:(cc + 1) * P], rhs=grs[:],
                             start=True, stop=True)
            ab = const.tile([P, 2], F32, tag=f"ab{cc}", name=f"ab{cc}")
            nc.vector.tensor_mul(ab[:, 0:1], bc_ps[:, 1:2], gaff_sb[cc][:, 0:1])
            t2 = tmp.tile([P, 1], F32, tag="t2")
            nc.vector.tensor_mul(t2[:], bc_ps[:, 0:1], ab[:, 0:1])
            nc.vector.tensor_sub(ab[:, 1:2], gaff_sb[cc][:, 1:2], t2[:])
            ab_sb.append(ab)

        # ---- h_q = a*x_q + b  (bf16) ----
        hq_sb = []
        for cc in range(NCC):
            t = utp.tile([P, NQ], BF16, tag=f"hq{cc}", name=f"hq{cc}")
            nc.vector.tensor_scalar(
                out=t[:], in0=xq_sb[cc][:],
                scalar1=ab_sb[cc][:, 0:1], scalar2=ab_sb[cc][:, 1:2],
                op0=AL.mult, op1=AL.add)
            hq_sb.append(t)

        # ---- u = a*(Mqk @ h_q + cq)  (bf16) ----
        ut_sb = []
        for oc in range(NCC):
            t = utp.tile([P, NQ], BF16, tag=f"ut{oc}", name=f"ut{oc}")
            ut_sb.append(t)
        for ih in range(NIH):
            isl = slice(ih * 512, (ih + 1) * 512)
            for oc in range(NCC):
                ups = psA.tile([P, 512], F32, tag=f"a{oc}", name=f"ups{oc}")
                for cc in range(NCC):
                    nc.tensor.matmul(
                        out=ups[:],
                        lhsT=mt_sb[cc][:, oc * P:(oc + 1) * P],
                        rhs=hq_sb[cc][:, isl],
                        start=(cc == 0), stop=(cc == NCC - 1))
                nc.vector.tensor_scalar(
                    out=ut_sb[oc][:, isl], in0=ups[:],
                    scalar1=cvec_sb[oc][:, 0:1], scalar2=ab_sb[oc][:, 0:1],
                    op0=AL.add, op1=AL.mult)

        # ---- deferred consts: cb2 = W2 @ b + cpv, then scale w2t by a ----
        cb2 = const.tile([P, NCC], F32, tag="cb2")

        def emit_cb2():
            bcol = const.tile([P, NCC], BF16, tag="bcol")
            for cc in range(NCC):
                nc.vector.tensor_copy(bcol[:, cc:cc + 1], ab_sb[cc][:, 1:2])
            for oc in range(NCC):
                ps = psE.tile([P, 1], F32, tag="e")
                for cc in range(NCC):
                    nc.tensor.matmul(
                        out=ps[:],
                        lhsT=w2t_sb[cc][:, oc * P:(oc + 1) * P],
                        rhs=bcol[:, cc:cc + 1],
                        start=(cc == 0), stop=(cc == NCC - 1))
                nc.vector.scalar_tensor_tensor(
                    out=cb2[:, oc:oc + 1], in0=cvec_sb[oc][:, 1:2],
                    scalar=1.0, in1=ps[:], op0=AL.mult, op1=AL.add)

        def emit_w2scale():
            for cc in range(NCC):
                nc.vector.tensor_scalar(
                    out=w2t_sb[cc][:], in0=w2t_sb[cc][:],
                    scalar1=ab_sb[cc][:, 0:1], scalar2=None, op0=AL.mult)

        # ---- attention j-loop, software-pipelined scores ----
        pts = {}

        def emit_S(ih, jb):
            isl = slice(ih * 512, (ih + 1) * 512)
            qq, jo = jb // 8, (jb % 8) * P
            S = psS.tile([P, 512], F32, tag="s")
            for cc in range(NCC):
                nc.tensor.matmul(
                    out=S[:],
                    lhsT=xh_sb[cc, qq][:, jo:jo + P],
                    rhs=ut_sb[cc][:, isl],
                    start=(cc == 0), stop=(cc == NCC - 1))
            pt = ptp.tile([P, 512], BF16, tag="pt")
            nc.scalar.activation(out=pt[:], in_=S[:], func=AF.Exp)
            pts[ih, jb] = pt

        def emit_LA(ih, jb, A, lp):
            pt = pts.pop((ih, jb))
            nc.tensor.matmul(out=lp[:], lhsT=ones_col[:], rhs=pt[:],
                             start=(jb == 0), stop=(jb == NJB - 1))
            for cv in range(NCC):
                nc.tensor.matmul(
                    out=A[cv][:],
                    lhsT=xt_sb[jb][:, cv * P:(cv + 1) * P],
                    rhs=pt[:],
                    start=(jb == 0), stop=(jb == NJB - 1))

        def emit_epilogue(ih, A, lp, interleave=()):
            isl = slice(ih * 512, (ih + 1) * 512)
            for thunk in interleave:
                thunk()
            lsb = tmp.tile([1, 512], F32, tag="lsb")
            nc.vector.tensor_copy(lsb[:], lp[:])
            lb_ps = psE.tile([P, 512], F32, tag="e")
            nc.tensor.matmul(out=lb_ps[:], lhsT=ones_row[:], rhs=lsb[:],
                             start=True, stop=True)
            rlb = tmp.tile([P, 512], F32, tag="rlb")
            nc.vector.reciprocal(rlb[:], lb_ps[:])
            Al = []
            for cv in range(NCC):
                t = alp.tile([P, 512], BF16, tag=f"al{cv}", name=f"al{cv}")
                nc.vector.tensor_mul(t[:], A[cv][:], rlb[:])
                Al.append(t)
            for oc in range(NCC):
                fps = psA.tile([P, 512], F32, tag=f"a{oc}", name=f"fps{oc}")
                for cc in range(NCC):
                    nc.tensor.matmul(
                        out=fps[:],
                        lhsT=w2t_sb[cc][:, oc * P:(oc + 1) * P],
                        rhs=Al[cc][:],
                        start=(cc == 0), stop=(cc == NCC - 1))
                fin = tmp.tile([P, 512], F32, tag="fin")
                nc.vector.scalar_tensor_tensor(
                    out=fin[:], in0=fps[:], scalar=cb2[:, oc:oc + 1],
                    in1=xq_sb[oc][:, isl], op0=AL.add, op1=AL.add)
                nc.sync.dma_start(out=out[oc * P:(oc + 1) * P, isl], in_=fin[:])

        def alloc_acc(ih):
            A = []
            for cv in range(NCC):
                t = psA.tile([P, 512], F32, tag=f"a{cv}", name=f"a{cv}")
                A.append(t)
            lp = psL.tile([1, 512], F32, tag="l")
            return A, lp

        A0, lp0 = alloc_acc(0)
        emit_S(0, 0)
        for jb in range(NJB):
            if jb + 1 < NJB:
                emit_S(0, jb + 1)
            emit_LA(0, jb, A0, lp0)
            if jb == 1:
                emit_cb2()
            if jb == 3:
                emit_w2scale()
        A1, lp1 = alloc_acc(1)
        emit_epilogue(0, A0, lp0,
                      interleave=(lambda: emit_S(1, 0), lambda: emit_S(1, 1)))
        for jb in range(NJB):
            if jb + 1 < NJB and (1, jb + 1) not in pts:
                emit_S(1, jb + 1)
            emit_LA(1, jb, A1, lp1)
        emit_epilogue(1, A1, lp1)

    nc.compile()
    return nc


_NC = None


def _get_nc():
    global _NC
    if _NC is None:
        _NC = build_nc()
    return _NC


def make_in_maps(x, gn_scale, gn_bias, wq, bq, wk, bk, wv, bv, wp, bp):
    f = np.float32
    d = np.float64
    x = np.asarray(x, f)
    wq = np.asarray(wq, f); wk = np.asarray(wk, f)
    wv = np.asarray(wv, f); wp = np.asarray(wp, f)
    bq = np.asarray(bq, f); bk = np.asarray(bk, f)
    bv = np.asarray(bv, f); bp = np.asarray(bp, f)
    gn_scale = np.asarray(gn_scale, f); gn_bias = np.asarray(gn_bias, f)

    # lhsT for u-projection: (Mqk)^T = scale * wq^T wk
    mt_np = np.ascontiguousarray(
        (SCALE * (wq.T.astype(d) @ wk.astype(d))).astype(f)).astype(BF)
    # lhsT for output projection: (wp wv)^T
    w2t_np = np.ascontiguousarray(
        (wp.astype(d) @ wv.astype(d)).T.astype(f)).astype(BF)
    cq_np = (SCALE * (wk.T.astype(d) @ bq.astype(d))).astype(f)
    cpv_np = (wp.astype(d) @ bv.astype(d) + bp).astype(f)
    cvec = np.ascontiguousarray(np.stack([cq_np, cpv_np], axis=1), f)
    gaff = np.ascontiguousarray(np.stack([gn_scale, gn_bias], axis=1), f)
    gmat = np.zeros((C, G), f)
    gmat[np.arange(C), np.arange(C) // (C // G)] = 1.0 / (C // G)
    gmatt = np.zeros((G, C), f)
    gmatt[np.arange(C) // (C // G), np.arange(C)] = 1.0

    in_maps = []
    for b in range(B):
        xb = np.ascontiguousarray(x[b].reshape(C, N))
        xh_b = xb.astype(BF)
        xt_b = np.ascontiguousarray(xb.T).astype(BF)
        for qc in range(N // NQ):
            xqc = np.ascontiguousarray(xb[:, qc * NQ:(qc + 1) * NQ])
            in_maps.append(dict(
                xh=xh_b, xt=xt_b, xq=xqc, mt=mt_np, w2t=w2t_np,
                cvec=cvec, gaff=gaff, gm=gmat, gmt=gmatt))
    return in_maps


def assemble(results, x):
    outf = np.empty((B, C, N), np.float32)
    i = 0
    for b in range(B):
        for qc in range(N // NQ):
            outf[b, :, qc * NQ:(qc + 1) * NQ] = results[i]["out"]
            i += 1
    return outf.reshape(x.shape)


def kernel(x, gn_scale, gn_bias, wq, bq, wk, bk, wv, bv, wp, bp, **run_kwargs):
    nc = _get_nc()
    in_maps = make_in_maps(x, gn_scale, gn_bias, wq, bq, wk, bk, wv, bv, wp, bp)
    res = run_bass_kernel_spmd(nc, in_maps, core_ids=list(range(8)), **run_kwargs)
    out = assemble(res.results, np.asarray(x))
    if run_kwargs:
        return out, res
    return out


# revision 7
# speedup vs baseline: 1.1665x; 1.1665x over previous
"""AttnBlock (GroupNorm + single-head self-attention + residual) for TRN2.

8 cores = 2 batches x 4 query-chunks of 1024 tokens.

v3 math restructure ("two-matrix form"): softmax is invariant to per-query
additive constants, and the per-token 1/l commutes with the output
projection.  Folding those out, the whole block needs only two host-
precomputed CxC matrices applied to RAW x:

  scores_ij ~ u_i . x_j   (mod per-i consts), u = a*(Mqk @ h_q + cq),
      Mqk = scale * wk^T wq,  h_q = a*x_q + b (GroupNorm affine)
  A_i = sum_j exp(s_ij) x_j ,  l_i = sum_j exp(s_ij)
  out = x + (WpWv*diag(a)) @ (A/l) + [WpWv b + wp bv + bp]

So K/V/Q/P projections of the token stream disappear (6.98 -> 4.83 GMAC
per core) and the attention matmuls consume x directly in bf16 (half the
DMA, FWL weight loads).  PSUM: A 4 banks (chain over all 32 j-blocks),
scores 2, l 1, epilogue 1 = 8.
"""

import numpy as np
import ml_dtypes
from contextlib import ExitStack

import concourse.bass as bass
import concourse.bacc as bacc
import concourse.tile as tile
from concourse import mybir
from concourse.bass_utils import run_bass_kernel_spmd

F32 = mybir.dt.float32
BF16 = mybir.dt.bfloat16
AL = mybir.AluOpType
AF = mybir.ActivationFunctionType

B = 2
C = 512
N = 4096
NQ = 1024
P = 128
NCC = C // P      # 4
G = 32
EPS = 1e-6
NJB = N // P      # 32 j-blocks of 128 tokens
NIH = NQ // 512   # 2 query halves of 512
SCALE = float(C) ** -0.5
BF = ml_dtypes.bfloat16


def build_nc():
    nc = bacc.Bacc(None, target_bir_lowering=False)

    xh = nc.dram_tensor("xh", [C, N], BF16, kind="ExternalInput")
    xt = nc.dram_tensor("xt", [N, C], BF16, kind="ExternalInput")
    xq = nc.dram_tensor("xq", [C, NQ], F32, kind="ExternalInput")
    mt = nc.dram_tensor("mt", [C, C], BF16, kind="ExternalInput")    # (scale*wk^T wq)^T
    w2t = nc.dram_tensor("w2t", [C, C], BF16, kind="ExternalInput")  # (wp wv)^T
    cvec = nc.dram_tensor("cvec", [C, 2], F32, kind="ExternalInput")  # [cq, wp@bv+bp]
    gaff = nc.dram_tensor("gaff", [C, 2], F32, kind="ExternalInput")
    gm = nc.dram_tensor("gm", [C, G], F32, kind="ExternalInput")     # indicator/16
    gmt = nc.dram_tensor("gmt", [G, C], F32, kind="ExternalInput")   # indicator
    out = nc.dram_tensor("out", [C, NQ], F32, kind="ExternalOutput")

    with tile.TileContext(nc) as tc, ExitStack() as ctx:
        const = ctx.enter_context(tc.tile_pool(name="const", bufs=1))
        xhp = ctx.enter_context(tc.tile_pool(name="xhp", bufs=1))
        xtp = ctx.enter_context(tc.tile_pool(name="xtp", bufs=1))
        xqp = ctx.enter_context(tc.tile_pool(name="xqp", bufs=1))
        wp_ = ctx.enter_context(tc.tile_pool(name="wp", bufs=1))
        utp = ctx.enter_context(tc.tile_pool(name="utp", bufs=1))
        ptp = ctx.enter_context(tc.tile_pool(name="ptp", bufs=3))
        alp = ctx.enter_context(tc.tile_pool(name="alp", bufs=1))
        tmp = ctx.enter_context(tc.tile_pool(name="tmp", bufs=2))
        psA = ctx.enter_context(tc.tile_pool(name="psA", bufs=1, space="PSUM"))
        psS = ctx.enter_context(tc.tile_pool(name="psS", bufs=2, space="PSUM"))
        psL = ctx.enter_context(tc.tile_pool(name="psL", bufs=1, space="PSUM"))
        psE = ctx.enter_context(tc.tile_pool(name="psE", bufs=1, space="PSUM"))

        # ---- tiny constant tables ----
        cvec_sb = []
        gaff_sb = []
        gm_sb = []
        for cc in range(NCC):
            t = const.tile([P, 2], F32, tag=f"cv{cc}", name=f"cv{cc}")
            nc.sync.dma_start(out=t[:], in_=cvec[cc * P:(cc + 1) * P, :])
            cvec_sb.append(t)
            t = const.tile([P, 2], F32, tag=f"ga{cc}", name=f"ga{cc}")
            nc.sync.dma_start(out=t[:], in_=gaff[cc * P:(cc + 1) * P, :])
            gaff_sb.append(t)
            t = const.tile([P, G], F32, tag=f"gm{cc}", name=f"gm{cc}")
            nc.sync.dma_start(out=t[:], in_=gm[cc * P:(cc + 1) * P, :])
            gm_sb.append(t)
        gmt_sb = const.tile([G, C], F32, tag="gmt")
        nc.sync.dma_start(out=gmt_sb[:], in_=gmt[:, :])
        eps_sb = const.tile([G, 1], F32, tag="eps")
        nc.vector.memset(eps_sb[:], EPS)
        ones_row = const.tile([1, P], F32, tag="onesr")
        nc.vector.memset(ones_row[:], 1.0)
        ones_col = const.tile([P, 1], BF16, tag="onesc")
        nc.vector.memset(ones_col[:], 1.0)
        # ---- DMA routing: critical path (mt, xh) exclusive on the SP HWDGE
        # queue; xq on the ACT HWDGE queue (4 cheap dispatches, ACT idle
        # until the first exp); bulk w2t/xt on the gpsimd software DGE so
        # neither critical queue nor the ACT engine is loaded. ----
        mt_sb = []
        for cc in range(NCC):
            t = wp_.tile([P, C], BF16, tag=f"mt{cc}", name=f"mt{cc}")
            nc.sync.dma_start(out=t[:], in_=mt[cc * P:(cc + 1) * P, :])
            mt_sb.append(t)
        xq_sb = []
        for cc in range(NCC):
            t = xqp.tile([P, NQ], F32, tag=f"xq{cc}", name=f"xq{cc}")
            nc.scalar.dma_start(out=t[:], in_=xq[cc * P:(cc + 1) * P, :])
            xq_sb.append(t)
        w2t_sb = []
        for cc in range(NCC):
            t = wp_.tile([P, C], BF16, tag=f"w2t{cc}", name=f"w2t{cc}")
            nc.gpsimd.dma_start(out=t[:], in_=w2t[cc * P:(cc + 1) * P, :])
            w2t_sb.append(t)

        # ---- x (bf16) quarter tiles: GroupNorm stats stream behind DMA ----
        NQT = N // 4
        xh_sb = {}
        for cc in range(NCC):
            for qq in range(4):
                t = xhp.tile([P, NQT], BF16, tag=f"xh{cc}{qq}", name=f"xh{cc}{qq}")
                nc.sync.dma_start(
                    out=t[:],
                    in_=xh[cc * P:(cc + 1) * P, qq * NQT:(qq + 1) * NQT])
                xh_sb[cc, qq] = t

        xt_sb = []
        for jb in range(NJB):
            t = xtp.tile([P, C], BF16, tag=f"xt{jb}", name=f"xt{jb}")
            nc.gpsimd.dma_start(out=t[:], in_=xt[jb * P:(jb + 1) * P, :])
            xt_sb.append(t)

        # ---- GroupNorm stats -> per-channel a, b ----
        mus = []
        for cc in range(NCC):
            stats = tmp.tile([P, 8, 6], F32, tag="bst")
            for qq in range(4):
                xv = xh_sb[cc, qq].rearrange("p (s f) -> p s f", f=512)
                for s in range(2):
                    nc.vector.bn_stats(out=stats[:, qq * 2 + s, :], in_=xv[:, s, :])
            mv = tmp.tile([P, 2], F32, tag="mv")
            nc.vector.bn_aggr(out=mv[:], in_=stats[:])
            mu = tmp.tile([P, 2], F32, tag=f"mu{cc}")
            nc.vector.tensor_copy(mu[:, 0:1], mv[:, 0:1])
            nc.vector.scalar_tensor_tensor(
                out=mu[:, 1:2], in0=mv[:, 0:1], scalar=mv[:, 0:1],
                in1=mv[:, 1:2], op0=AL.mult, op1=AL.add)
            mus.append(mu)
        agg_ps = psE.tile([G, 2], F32, tag="e")
        for cc in range(NCC):
            nc.tensor.matmul(out=agg_ps[:], lhsT=gm_sb[cc][:], rhs=mus[cc][:],
                             start=(cc == 0), stop=(cc == NCC - 1))
        eg = tmp.tile([G, 2], F32, tag="eg")
        nc.vector.tensor_copy(eg[:], agg_ps[:])
        msq = tmp.tile([G, 1], F32, tag="msq")
        nc.vector.tensor_mul(msq[:], eg[:, 0:1], eg[:, 0:1])
        grs = tmp.tile([G, 2], F32, tag="grs")
        nc.vector.tensor_copy(grs[:, 0:1], eg[:, 0:1])
        var = tmp.tile([G, 1], F32, tag="var")
        nc.vector.tensor_sub(var[:], eg[:, 1:2], msq[:])
        std = tmp.tile([G, 1], F32, tag="std")
        nc.scalar.activation(out=std[:], in_=var[:], func=AF.Sqrt, bias=eps_sb[:])
        nc.vector.reciprocal(grs[:, 1:2], std[:])

        ab_sb = []
        for cc in range(NCC):
            bc_ps = psE.tile([P, 2], F32, tag="e")
            nc.tensor.matmul(out=bc_ps[:],
                             lhsT=gmt_sb[:, cc * P:(cc + 1) * P], rhs=grs[:],
                             start=True, stop=True)
            ab = const.tile([P, 2], F32, tag=f"ab{cc}", name=f"ab{cc}")
            nc.vector.tensor_mul(ab[:, 0:1], bc_ps[:, 1:2], gaff_sb[cc][:, 0:1])
            t2 = tmp.tile([P, 1], F32, tag="t2")
            nc.vector.tensor_mul(t2[:], bc_ps[:, 0:1], ab[:, 0:1])
            nc.vector.tensor_sub(ab[:, 1:2], gaff_sb[cc][:, 1:2], t2[:])
            ab_sb.append(ab)

        # ---- h_q = a*x_q + b  (bf16) ----
        hq_sb = []
        for cc in range(NCC):
            t = utp.tile([P, NQ], BF16, tag=f"hq{cc}", name=f"hq{cc}")
            nc.vector.tensor_scalar(
                out=t[:], in0=xq_sb[cc][:],
                scalar1=ab_sb[cc][:, 0:1], scalar2=ab_sb[cc][:, 1:2],
                op0=AL.mult, op1=AL.add)
            hq_sb.append(t)

        # ---- u = a*(Mqk @ h_q + cq)  (bf16) ----
        ut_sb = []
        for oc in range(NCC):
            t = utp.tile([P, NQ], BF16, tag=f"ut{oc}", name=f"ut{oc}")
            ut_sb.append(t)
        for ih in range(NIH):
            isl = slice(ih * 512, (ih + 1) * 512)
            for oc in range(NCC):
                ups = psA.tile([P, 512], F32, tag=f"a{oc}", name=f"ups{oc}")
                for cc in range(NCC):
                    nc.tensor.matmul(
                        out=ups[:],
                        lhsT=mt_sb[cc][:, oc * P:(oc + 1) * P],
                        rhs=hq_sb[cc][:, isl],
                        start=(cc == 0), stop=(cc == NCC - 1))
                nc.vector.tensor_scalar(
                    out=ut_sb[oc][:, isl], in0=ups[:],
                    scalar1=cvec_sb[oc][:, 0:1], scalar2=ab_sb[oc][:, 0:1],
                    op0=AL.add, op1=AL.mult)

        # ---- deferred consts: cb2 = W2 @ b + cpv, then scale w2t by a ----
        cb2 = const.tile([P, NCC], F32, tag="cb2")

        def emit_cb2():
            bcol = const.tile([P, NCC], BF16, tag="bcol")
            for cc in range(NCC):
                nc.vector.tensor_copy(bcol[:, cc:cc + 1], ab_sb[cc][:, 1:2])
            for oc in range(NCC):
                ps = psE.tile([P, 1], F32, tag="e")
                for cc in range(NCC):
                    nc.tensor.matmul(
                        out=ps[:],
                        lhsT=w2t_sb[cc][:, oc * P:(oc + 1) * P],
                        rhs=bcol[:, cc:cc + 1],
                        start=(cc == 0), stop=(cc == NCC - 1))
                nc.vector.scalar_tensor_tensor(
                    out=cb2[:, oc:oc + 1], in0=cvec_sb[oc][:, 1:2],
                    scalar=1.0, in1=ps[:], op0=AL.mult, op1=AL.add)

        def emit_w2scale():
            for cc in range(NCC):
                nc.vector.tensor_scalar(
                    out=w2t_sb[cc][:], in0=w2t_sb[cc][:],
                    scalar1=ab_sb[cc][:, 0:1], scalar2=None, op0=AL.mult)

        # ---- attention j-loop, software-pipelined scores ----
        pts = {}

        def emit_S(ih, jb):
            isl = slice(ih * 512, (ih + 1) * 512)
            qq, jo = jb // 8, (jb % 8) * P
            S = psS.tile([P, 512], F32, tag="s")
            for cc in range(NCC):
                nc.tensor.matmul(
                    out=S[:],
                    lhsT=xh_sb[cc, qq][:, jo:jo + P],
                    rhs=ut_sb[cc][:, isl],
                    start=(cc == 0), stop=(cc == NCC - 1))
            pt = ptp.tile([P, 512], BF16, tag="pt")
            nc.scalar.activation(out=pt[:], in_=S[:], func=AF.Exp)
            pts[ih, jb] = pt

        def emit_LA(ih, jb, A, lp):
            pt = pts.pop((ih, jb))
            nc.tensor.matmul(out=lp[:], lhsT=ones_col[:], rhs=pt[:],
                             start=(jb == 0), stop=(jb == NJB - 1))
            for cv in range(NCC):
                nc.tensor.matmul(
                    out=A[cv][:],
                    lhsT=xt_sb[jb][:, cv * P:(cv + 1) * P],
                    rhs=pt[:],
                    start=(jb == 0), stop=(jb == NJB - 1))

        def emit_epilogue(ih, A, lp, interleave=()):
            # W2 @ (A/l) == (W2 @ A)/l (per-token scalar commutes with the
            # left matmul), so the W2 matmuls start right off the raw A and
            # the division folds into the epilogue DVE pass.
            isl = slice(ih * 512, (ih + 1) * 512)
            for thunk in interleave:
                thunk()
            lsb = tmp.tile([1, 512], F32, tag="lsb")
            nc.vector.tensor_copy(lsb[:], lp[:])
            Al = []
            for cv in range(NCC):
                t = alp.tile([P, 512], BF16, tag=f"al{cv}", name=f"al{cv}")
                nc.vector.tensor_copy(t[:], A[cv][:])
                Al.append(t)
            lb_ps = psE.tile([P, 512], F32, tag="e")
            nc.tensor.matmul(out=lb_ps[:], lhsT=ones_row[:], rhs=lsb[:],
                             start=True, stop=True)
            rlb = tmp.tile([P, 512], F32, tag="rlb")
            nc.vector.reciprocal(rlb[:], lb_ps[:])
            for oc in range(NCC):
                fps = psA.tile([P, 512], F32, tag=f"a{oc}", name=f"fps{oc}")
                for cc in range(NCC):
                    nc.tensor.matmul(
                        out=fps[:],
                        lhsT=w2t_sb[cc][:, oc * P:(oc + 1) * P],
                        rhs=Al[cc][:],
                        start=(cc == 0), stop=(cc == NCC - 1))
                ft = tmp.tile([P, 512], F32, tag="ft")
                nc.vector.tensor_mul(ft[:], fps[:], rlb[:])
                fin = tmp.tile([P, 512], F32, tag="fin")
                nc.vector.scalar_tensor_tensor(
                    out=fin[:], in0=ft[:], scalar=cb2[:, oc:oc + 1],
                    in1=xq_sb[oc][:, isl], op0=AL.add, op1=AL.add)
                nc.sync.dma_start(out=out[oc * P:(oc + 1) * P, isl], in_=fin[:])

        def alloc_acc(ih):
            A = []
            for cv in range(NCC):
                t = psA.tile([P, 512], F32, tag=f"a{cv}", name=f"a{cv}")
                A.append(t)
            lp = psL.tile([1, 512], F32, tag="l")
            return A, lp

        A0, lp0 = alloc_acc(0)
        emit_S(0, 0)
        for jb in range(NJB):
            if jb + 1 < NJB:
                emit_S(0, jb + 1)
            emit_LA(0, jb, A0, lp0)
            if jb == 1:
                emit_cb2()
            if jb == 3:
                emit_w2scale()
        A1, lp1 = alloc_acc(1)
        emit_epilogue(0, A0, lp0,
                      interleave=(lambda: emit_S(1, 0), lambda: emit_S(1, 1)))
        for jb in range(NJB):
            if jb + 1 < NJB and (1, jb + 1) not in pts:
                emit_S(1, jb + 1)
            emit_LA(1, jb, A1, lp1)
        emit_epilogue(1, A1, lp1)

    nc.compile()
    return nc


_NC = None


def _get_nc():
    global _NC
    if _NC is None:
        _NC = build_nc()
    return _NC


def make_in_maps(x, gn_scale, gn_bias, wq, bq, wk, bk, wv, bv, wp, bp):
    f = np.float32
    d = np.float64
    x = np.asarray(x, f)
    wq = np.asarray(wq, f); wk = np.asarray(wk, f)
    wv = np.asarray(wv, f); wp = np.asarray(wp, f)
    bq = np.asarray(bq, f); bk = np.asarray(bk, f)
    bv = np.asarray(bv, f); bp = np.asarray(bp, f)
    gn_scale = np.asarray(gn_scale, f); gn_bias = np.asarray(gn_bias, f)

    # lhsT for u-projection: (Mqk)^T = scale * wq^T wk
    mt_np = np.ascontiguousarray(
        (SCALE * (wq.T.astype(d) @ wk.astype(d))).astype(f)).astype(BF)
    # lhsT for output projection: (wp wv)^T
    w2t_np = np.ascontiguousarray(
        (wp.astype(d) @ wv.astype(d)).T.astype(f)).astype(BF)
    cq_np = (SCALE * (wk.T.astype(d) @ bq.astype(d))).astype(f)
    cpv_np = (wp.astype(d) @ bv.astype(d) + bp).astype(f)
    cvec = np.ascontiguousarray(np.stack([cq_np, cpv_np], axis=1), f)
    gaff = np.ascontiguousarray(np.stack([gn_scale, gn_bias], axis=1), f)
    gmat = np.zeros((C, G), f)
    gmat[np.arange(C), np.arange(C) // (C // G)] = 1.0 / (C // G)
    gmatt = np.zeros((G, C), f)
    gmatt[np.arange(C) // (C // G), np.arange(C)] = 1.0

    in_maps = []
    for b in range(B):
        xb = np.ascontiguousarray(x[b].reshape(C, N))
        xh_b = xb.astype(BF)
        xt_b = np.ascontiguousarray(xb.T).astype(BF)
        for qc in range(N // NQ):
            xqc = np.ascontiguousarray(xb[:, qc * NQ:(qc + 1) * NQ])
            in_maps.append(dict(
                xh=xh_b, xt=xt_b, xq=xqc, mt=mt_np, w2t=w2t_np,
                cvec=cvec, gaff=gaff, gm=gmat, gmt=gmatt))
    return in_maps


def assemble(results, x):
    outf = np.empty((B, C, N), np.float32)
    i = 0
    for b in range(B):
        for qc in range(N // NQ):
            outf[b, :, qc * NQ:(qc + 1) * NQ] = results[i]["out"]
            i += 1
    return outf.reshape(x.shape)


def kernel(x, gn_scale, gn_bias, wq, bq, wk, bk, wv, bv, wp, bp, **run_kwargs):
    nc = _get_nc()
    in_maps = make_in_maps(x, gn_scale, gn_bias, wq, bq, wk, bk, wv, bv, wp, bp)
    res = run_bass_kernel_spmd(nc, in_maps, core_ids=list(range(8)), **run_kwargs)
    out = assemble(res.results, np.asarray(x))
    if run_kwargs:
        return out, res
    return out


# revision 12
# speedup vs baseline: 1.9950x; 1.7103x over previous
"""AttnBlock (GroupNorm + single-head self-attention + residual) for TRN2.

8 cores = 2 batches x 4 query-chunks of 1024 tokens.

v6: "two-matrix" math restructure + fp8 DoubleRow attention.

Math: softmax is invariant to per-query additive constants, and the
per-token 1/l commutes with the output projection, so the block needs just
two host-precomputed CxC matrices applied to RAW x:

  scores_ij ~ u_i . x_j   (mod per-i consts), u = a*(Mqk @ h_q + cq),
      Mqk = scale * wk^T wq,  h_q = a*x_q + b (GroupNorm affine)
  A_i = sum_j p_ij x_j ,  l_i = sum_j p_ij ,  p = exp(s - 4)  (the -4
      keeps p in e4m3 range; the constant cancels in A/l)
  out = x + (WpWv*diag(a)) @ A/l + [WpWv b + wp bv + bp]

The two big matmul chains (scores, PV) run in fp8e4 with
perf_mode=DoubleRow: operands are [128, 2, free] pair-tiles so each MM
contracts 256 (2 chunks).  Everything else is bf16/fp32.  Bulk inputs are
host-pre-tiled to contiguous blocks and DMA'd via the gpsimd software DGE
(which aggregates descriptors; the HWDGE queues crawl on strided tiles).
PSUM: A 4 banks (chain over all 16 j-pairs), scores 2, l 1, epilogue 1.
"""

import numpy as np
import ml_dtypes
from contextlib import ExitStack

import concourse.bass as bass
import concourse.bacc as bacc
import concourse.tile as tile
from concourse import mybir
from concourse.bass_utils import run_bass_kernel_spmd

F32 = mybir.dt.float32
BF16 = mybir.dt.bfloat16
FP8 = mybir.dt.float8e4
AL = mybir.AluOpType
AF = mybir.ActivationFunctionType
DR = mybir.MatmulPerfMode.DoubleRow

B = 2
C = 512
N = 4096
NQ = 1024
P = 128
NCC = C // P      # 4 channel chunks
NCP = NCC // 2    # 2 channel pairs
G = 32
EPS = 1e-6
NJB = N // P      # 32 j-blocks
NJP = NJB // 2    # 16 j-pairs
NIH = NQ // 512   # 2 query halves
SCALE = float(C) ** -0.5
BF = ml_dtypes.bfloat16
F8 = ml_dtypes.float8_e4m3
EXP_BIAS = -4.0


def build_nc():
    nc = bacc.Bacc(None, target_bir_lowering=False)

    # x in fp8, pre-tiled: xh8[cp][p, k, n] = x[(2cp+k)*128+p, n]
    xh8 = nc.dram_tensor("xh8", [NCP, P, 2, N], FP8, kind="ExternalInput")
    # x^T in fp8, pre-tiled: xt8[jp][p, k, c] = x[c, (2jp+k)*128+p]
    xt8 = nc.dram_tensor("xt8", [NJP, P, 2, C], FP8, kind="ExternalInput")
    xq = nc.dram_tensor("xq", [C, NQ], F32, kind="ExternalInput")
    mt = nc.dram_tensor("mt", [C, C], BF16, kind="ExternalInput")    # (scale*wk^T wq)^T
    w2t = nc.dram_tensor("w2t", [C, C], BF16, kind="ExternalInput")  # (wp wv)^T
    cvec = nc.dram_tensor("cvec", [C, 2], F32, kind="ExternalInput")  # [cq, wp@bv+bp]
    gaff = nc.dram_tensor("gaff", [C, 2], F32, kind="ExternalInput")
    gm = nc.dram_tensor("gm", [C, G], F32, kind="ExternalInput")     # indicator/16
    gmt = nc.dram_tensor("gmt", [G, C], F32, kind="ExternalInput")   # indicator
    out = nc.dram_tensor("out", [NIH, NCC, P, 512], F32, kind="ExternalOutput")

    with tile.TileContext(nc) as tc, ExitStack() as ctx:
        const = ctx.enter_context(tc.tile_pool(name="const", bufs=1))
        xhp = ctx.enter_context(tc.tile_pool(name="xhp", bufs=1))
        xtp = ctx.enter_context(tc.tile_pool(name="xtp", bufs=1))
        xqp = ctx.enter_context(tc.tile_pool(name="xqp", bufs=1))
        wp_ = ctx.enter_context(tc.tile_pool(name="wp", bufs=1))
        utp = ctx.enter_context(tc.tile_pool(name="utp", bufs=1))
        ptp = ctx.enter_context(tc.tile_pool(name="ptp", bufs=2))
        alp = ctx.enter_context(tc.tile_pool(name="alp", bufs=1))
        tmp = ctx.enter_context(tc.tile_pool(name="tmp", bufs=2))
        psA = ctx.enter_context(tc.tile_pool(name="psA", bufs=1, space="PSUM"))
        psS = ctx.enter_context(tc.tile_pool(name="psS", bufs=2, space="PSUM"))
        psL = ctx.enter_context(tc.tile_pool(name="psL", bufs=1, space="PSUM"))
        psE = ctx.enter_context(tc.tile_pool(name="psE", bufs=1, space="PSUM"))

        # ---- tiny constant tables (SP HWDGE queue) ----
        cvec_sb = []
        gaff_sb = []
        gm_sb = []
        for cc in range(NCC):
            t = const.tile([P, 2], F32, tag=f"cv{cc}", name=f"cv{cc}")
            nc.sync.dma_start(out=t[:], in_=cvec[cc * P:(cc + 1) * P, :])
            cvec_sb.append(t)
            t = const.tile([P, 2], F32, tag=f"ga{cc}", name=f"ga{cc}")
            nc.sync.dma_start(out=t[:], in_=gaff[cc * P:(cc + 1) * P, :])
            gaff_sb.append(t)
            t = const.tile([P, G], F32, tag=f"gm{cc}", name=f"gm{cc}")
            nc.sync.dma_start(out=t[:], in_=gm[cc * P:(cc + 1) * P, :])
            gm_sb.append(t)
        gmt_sb = const.tile([G, C], F32, tag="gmt")
        nc.sync.dma_start(out=gmt_sb[:], in_=gmt[:, :])
        eps_sb = const.tile([G, 1], F32, tag="eps")
        nc.vector.memset(eps_sb[:], EPS)
        ones_row = const.tile([1, P], F32, tag="onesr")
        nc.vector.memset(ones_row[:], 1.0)
        ones2 = const.tile([P, 2, 16], FP8, tag="ones2")
        nc.vector.memset(ones2[:], 1.0)
        ebias_sb = const.tile([P, 1], F32, tag="ebias")
        nc.vector.memset(ebias_sb[:], EXP_BIAS)

        # ---- bulk inputs on the gpsimd SWDGE, priority order ----
        xh8_sb = []
        for cp in range(NCP):
            t = xhp.tile([P, 2, N], FP8, tag=f"xh{cp}", name=f"xh{cp}")
            nc.gpsimd.dma_start(out=t[:], in_=xh8[cp])
            xh8_sb.append(t)
        xq_sb = []
        for cc in range(NCC):
            t = xqp.tile([P, NQ], F32, tag=f"xq{cc}", name=f"xq{cc}")
            nc.gpsimd.dma_start(out=t[:], in_=xq[cc * P:(cc + 1) * P, :])
            xq_sb.append(t)
        mt_sb = []
        for cc in range(NCC):
            t = wp_.tile([P, C], BF16, tag=f"mt{cc}", name=f"mt{cc}")
            nc.gpsimd.dma_start(out=t[:], in_=mt[cc * P:(cc + 1) * P, :])
            mt_sb.append(t)
        xt8_sb = []
        for jp in range(NJP):
            t = xtp.tile([P, 2, C], FP8, tag=f"xt{jp}", name=f"xt{jp}")
            nc.gpsimd.dma_start(out=t[:], in_=xt8[jp])
            xt8_sb.append(t)
        w2t_sb = []
        for cc in range(NCC):
            t = wp_.tile([P, C], BF16, tag=f"w2t{cc}", name=f"w2t{cc}")
            nc.gpsimd.dma_start(out=t[:], in_=w2t[cc * P:(cc + 1) * P, :])
            w2t_sb.append(t)

        # ---- GroupNorm stats (from the fp8 x) -> per-channel a, b ----
        mus = []
        for cc in range(NCC):
            xsl = xh8_sb[cc // 2][:, cc % 2, :].rearrange(
                "p (s f) -> p s f", f=512)
            stats = tmp.tile([P, 8, 6], F32, tag="bst")
            for s in range(8):
                nc.vector.bn_stats(out=stats[:, s, :], in_=xsl[:, s, :])
            mv = tmp.tile([P, 2], F32, tag="mv")
            nc.vector.bn_aggr(out=mv[:], in_=stats[:])
            mu = tmp.tile([P, 2], F32, tag=f"mu{cc}")
            nc.vector.tensor_copy(mu[:, 0:1], mv[:, 0:1])
            nc.vector.scalar_tensor_tensor(
                out=mu[:, 1:2], in0=mv[:, 0:1], scalar=mv[:, 0:1],
                in1=mv[:, 1:2], op0=AL.mult, op1=AL.add)
            mus.append(mu)
        agg_ps = psE.tile([G, 2], F32, tag="e")
        for cc in range(NCC):
            nc.tensor.matmul(out=agg_ps[:], lhsT=gm_sb[cc][:], rhs=mus[cc][:],
                             start=(cc == 0), stop=(cc == NCC - 1))
        eg = tmp.tile([G, 2], F32, tag="eg")
        nc.vector.tensor_copy(eg[:], agg_ps[:])
        msq = tmp.tile([G, 1], F32, tag="msq")
        nc.vector.tensor_mul(msq[:], eg[:, 0:1], eg[:, 0:1])
        grs = tmp.tile([G, 2], F32, tag="grs")
        nc.vector.tensor_copy(grs[:, 0:1], eg[:, 0:1])
        var = tmp.tile([G, 1], F32, tag="var")
        nc.vector.tensor_sub(var[:], eg[:, 1:2], msq[:])
        std = tmp.tile([G, 1], F32, tag="std")
        nc.scalar.activation(out=std[:], in_=var[:], func=AF.Sqrt, bias=eps_sb[:])
        nc.vector.reciprocal(grs[:, 1:2], std[:])

        ab_sb = []
        for cc in range(NCC):
            bc_ps = psE.tile([P, 2], F32, tag="e")
            nc.tensor.matmul(out=bc_ps[:],
                             lhsT=gmt_sb[:, cc * P:(cc + 1) * P], rhs=grs[:],
                             start=True, stop=True)
            ab = const.tile([P, 2], F32, tag=f"ab{cc}", name=f"ab{cc}")
            nc.vector.tensor_mul(ab[:, 0:1], bc_ps[:, 1:2], gaff_sb[cc][:, 0:1])
            t2 = tmp.tile([P, 1], F32, tag="t2")
            nc.vector.tensor_mul(t2[:], bc_ps[:, 0:1], ab[:, 0:1])
            nc.vector.tensor_sub(ab[:, 1:2], gaff_sb[cc][:, 1:2], t2[:])
            ab_sb.append(ab)

        # ---- h_q = a*x_q + b  (bf16) ----
        hq_sb = []
        for cc in range(NCC):
            t = utp.tile([P, NQ], BF16, tag=f"hq{cc}", name=f"hq{cc}")
            nc.vector.tensor_scalar(
                out=t[:], in0=xq_sb[cc][:],
                scalar1=ab_sb[cc][:, 0:1], scalar2=ab_sb[cc][:, 1:2],
                op0=AL.mult, op1=AL.add)
            hq_sb.append(t)

        # ---- u = a*(Mqk @ h_q + cq)  (fp8 pair-tiles for DoubleRow) ----
        ut8_sb = []
        for cp in range(NCP):
            t = utp.tile([P, 2, NQ], FP8, tag=f"ut{cp}", name=f"ut{cp}")
            ut8_sb.append(t)
        for ih in range(NIH):
            isl = slice(ih * 512, (ih + 1) * 512)
            for oc in range(NCC):
                ups = psA.tile([P, 512], F32, tag=f"a{oc}", name=f"ups{oc}")
                for cc in range(NCC):
                    nc.tensor.matmul(
                        out=ups[:],
                        lhsT=mt_sb[cc][:, oc * P:(oc + 1) * P],
                        rhs=hq_sb[cc][:, isl],
                        start=(cc == 0), stop=(cc == NCC - 1))
                nc.vector.tensor_scalar(
                    out=ut8_sb[oc // 2][:, oc % 2, isl], in0=ups[:],
                    scalar1=cvec_sb[oc][:, 0:1], scalar2=ab_sb[oc][:, 0:1],
                    op0=AL.add, op1=AL.mult)

        # ---- deferred consts: cb2 = W2 @ b + cpv, then scale w2t by a ----
        cb2 = const.tile([P, NCC], F32, tag="cb2")

        def emit_cb2():
            bcol = const.tile([P, NCC], BF16, tag="bcol")
            for cc in range(NCC):
                nc.vector.tensor_copy(bcol[:, cc:cc + 1], ab_sb[cc][:, 1:2])
            for oc in range(NCC):
                ps = psE.tile([P, 1], F32, tag="e")
                for cc in range(NCC):
                    nc.tensor.matmul(
                        out=ps[:],
                        lhsT=w2t_sb[cc][:, oc * P:(oc + 1) * P],
                        rhs=bcol[:, cc:cc + 1],
                        start=(cc == 0), stop=(cc == NCC - 1))
                nc.vector.scalar_tensor_tensor(
                    out=cb2[:, oc:oc + 1], in0=cvec_sb[oc][:, 1:2],
                    scalar=1.0, in1=ps[:], op0=AL.mult, op1=AL.add)

        def emit_w2scale():
            for cc in range(NCC):
                nc.vector.tensor_scalar(
                    out=w2t_sb[cc][:], in0=w2t_sb[cc][:],
                    scalar1=ab_sb[cc][:, 0:1], scalar2=None, op0=AL.mult)

        # ---- attention: fp8 DoubleRow, j in pairs, pipelined scores ----
        pt8 = {}

        def emit_S(ih, jb):
            isl = slice(ih * 512, (ih + 1) * 512)
            jo = (jb % NJB) * P
            S = psS.tile([P, 512], F32, tag="s")
            for cp in range(NCP):
                nc.tensor.matmul(
                    out=S[:],
                    lhsT=xh8_sb[cp][:, :, jo:jo + P],
                    rhs=ut8_sb[cp][:, :, isl],
                    start=(cp == 0), stop=(cp == NCP - 1),
                    perf_mode=DR)
            jp, k = jb // 2, jb % 2
            if k == 0:
                pt8[ih, jp] = ptp.tile([P, 2, 512], FP8, tag="pt", name="pt")
            nc.scalar.activation(out=pt8[ih, jp][:, k, :], in_=S[:],
                                 func=AF.Exp, bias=ebias_sb[:])

        def emit_LA(ih, jp, A, lp):
            pt = pt8.pop((ih, jp))
            nc.tensor.matmul(out=lp[:], lhsT=ones2[:, :, 0:1], rhs=pt[:],
                             start=(jp == 0), stop=(jp == NJP - 1),
                             perf_mode=DR)
            for cv in range(NCC):
                nc.tensor.matmul(
                    out=A[cv][:],
                    lhsT=xt8_sb[jp][:, :, cv * P:(cv + 1) * P],
                    rhs=pt[:],
                    start=(jp == 0), stop=(jp == NJP - 1),
                    perf_mode=DR)

        def emit_epilogue(ih, A, lp, interleave=()):
            # W2 @ (A/l) == (W2 @ A)/l: W2 matmuls start right off raw A,
            # the division folds into the epilogue DVE pass.
            isl = slice(ih * 512, (ih + 1) * 512)
            for thunk in interleave:
                thunk()
            lsb = tmp.tile([1, 512], F32, tag="lsb")
            nc.vector.tensor_copy(lsb[:], lp[:])
            Al = []
            for cv in range(NCC):
                t = alp.tile([P, 512], BF16, tag=f"al{cv}", name=f"al{cv}")
                nc.vector.tensor_copy(t[:], A[cv][:])
                Al.append(t)
            lb_ps = psE.tile([P, 512], F32, tag="e")
            nc.tensor.matmul(out=lb_ps[:], lhsT=ones_row[:], rhs=lsb[:],
                             start=True, stop=True)
            rlb = tmp.tile([P, 512], F32, tag="rlb")
            nc.vector.reciprocal(rlb[:], lb_ps[:])
            for oc in range(NCC):
                fps = psA.tile([P, 512], F32, tag=f"a{oc}", name=f"fps{oc}")
                for cc in range(NCC):
                    nc.tensor.matmul(
                        out=fps[:],
                        lhsT=w2t_sb[cc][:, oc * P:(oc + 1) * P],
                        rhs=Al[cc][:],
                        start=(cc == 0), stop=(cc == NCC - 1))
                ft = tmp.tile([P, 512], F32, tag="ft")
                nc.vector.tensor_mul(ft[:], fps[:], rlb[:])
                fin = tmp.tile([P, 512], F32, tag="fin")
                nc.vector.scalar_tensor_tensor(
                    out=fin[:], in0=ft[:], scalar=cb2[:, oc:oc + 1],
                    in1=xq_sb[oc][:, isl], op0=AL.add, op1=AL.add)
                nc.sync.dma_start(out=out[ih, oc], in_=fin[:])

        def alloc_acc():
            A = []
            for cv in range(NCC):
                t = psA.tile([P, 512], F32, tag=f"a{cv}", name=f"a{cv}")
                A.append(t)
            lp = psL.tile([1, 512], F32, tag="l")
            return A, lp

        A0, lp0 = alloc_acc()
        emit_S(0, 0)
        emit_S(0, 1)
        for jp in range(NJP):
            if jp + 1 < NJP:
                emit_S(0, 2 * jp + 2)
                emit_S(0, 2 * jp + 3)
            emit_LA(0, jp, A0, lp0)
            if jp == 1:
                emit_cb2()
            if jp == 2:
                emit_w2scale()
        A1, lp1 = alloc_acc()
        emit_epilogue(0, A0, lp0,
                      interleave=(lambda: emit_S(1, 0), lambda: emit_S(1, 1)))
        for jp in range(NJP):
            if jp + 1 < NJP:
                emit_S(1, 2 * jp + 2)
                emit_S(1, 2 * jp + 3)
            emit_LA(1, jp, A1, lp1)
        emit_epilogue(1, A1, lp1)

    nc.compile()
    return nc


_NC = None


def _get_nc():
    global _NC
    if _NC is None:
        _NC = build_nc()
    return _NC


def make_in_maps(x, gn_scale, gn_bias, wq, bq, wk, bk, wv, bv, wp, bp):
    f = np.float32
    d = np.float64
    x = np.asarray(x, f)
    wq = np.asarray(wq, f); wk = np.asarray(wk, f)
    wv = np.asarray(wv, f); wp = np.asarray(wp, f)
    bq = np.asarray(bq, f); bk = np.asarray(bk, f)
    bv = np.asarray(bv, f); bp = np.asarray(bp, f)
    gn_scale = np.asarray(gn_scale, f); gn_bias = np.asarray(gn_bias, f)

    mt_np = np.ascontiguousarray(
        (SCALE * (wq.T.astype(d) @ wk.astype(d))).astype(f)).astype(BF)
    w2t_np = np.ascontiguousarray(
        (wp.astype(d) @ wv.astype(d)).T.astype(f)).astype(BF)
    cq_np = (SCALE * (wk.T.astype(d) @ bq.astype(d))).astype(f)
    cpv_np = (wp.astype(d) @ bv.astype(d) + bp).astype(f)
    cvec = np.ascontiguousarray(np.stack([cq_np, cpv_np], axis=1), f)
    gaff = np.ascontiguousarray(np.stack([gn_scale, gn_bias], axis=1), f)
    gmat = np.zeros((C, G), f)
    gmat[np.arange(C), np.arange(C) // (C // G)] = 1.0 / (C // G)
    gmatt = np.zeros((G, C), f)
    gmatt[np.arange(C) // (C // G), np.arange(C)] = 1.0

    in_maps = []
    for b in range(B):
        xb = np.ascontiguousarray(x[b].reshape(C, N))
        x8 = xb.astype(F8)
        # xh8[cp, p, k, n] = x8[(2cp+k)*128+p, n]
        xh8_b = np.ascontiguousarray(
            x8.reshape(NCP, 2, P, N).transpose(0, 2, 1, 3))
        # xt8[jp, p, k, c] = x8[c, (2jp+k)*128+p]
        xt8_b = np.ascontiguousarray(
            x8.T.reshape(NJP, 2, P, C).transpose(0, 2, 1, 3))
        for qc in range(N // NQ):
            xqc = np.ascontiguousarray(xb[:, qc * NQ:(qc + 1) * NQ])
            in_maps.append(dict(
                xh8=xh8_b, xt8=xt8_b, xq=xqc, mt=mt_np, w2t=w2t_np,
                cvec=cvec, gaff=gaff, gm=gmat, gmt=gmatt))
    return in_maps


def assemble(results, x):
    outf = np.empty((B, C, N), np.float32)
    i = 0
    for b in range(B):
        for qc in range(N // NQ):
            o = results[i]["out"]  # [NIH, NCC, P, 512]
            o = o.transpose(1, 2, 0, 3).reshape(C, NQ)
            outf[b, :, qc * NQ:(qc + 1) * NQ] = o
            i += 1
    return outf.reshape(x.shape)


def kernel(x, gn_scale, gn_bias, wq, bq, wk, bk, wv, bv, wp, bp, **run_kwargs):
    nc = _get_nc()
    in_maps = make_in_maps(x, gn_scale, gn_bias, wq, bq, wk, bk, wv, bv, wp, bp)
    res = run_bass_kernel_spmd(nc, in_maps, core_ids=list(range(8)), **run_kwargs)
    out = assemble(res.results, np.asarray(x))
    if run_kwargs:
        return out, res
    return out
